# revision 11
# baseline (speedup 1.0000x reference)
"""GRU cell kernel for Trainium2, data-parallel over batch across 8 NeuronCores.

Reference computation (B=8192, D=H=1024), per batch row:
    z = sigmoid(inp@wz + state@uz + bz)
    r = sigmoid(inp@wr + state@ur + br)
    h_ = tanh(inp@wx + bx + (state@wh) * r)
    hid = (1-z)*h_ + state*z

Strategy: each core takes a 1024-row batch shard. The z/r projections fuse
into one [1024,2048]@[2048,2048] GEMM (act = [inp|state], W = [[wz,wr],[uz,ur]]).
xh and hh stay separate GEMMs ([1024,1024]@[1024,1024] each) because hh is
gated by r before the sum. Activations are shipped pre-transposed from the
host ([K,B] layout) so they can be the stationary matmul operand; weights
stream from HBM as the moving operand. Biases enter the PSUM accumulation
group as a K=1 rank-one matmul against a ones-row. A fused DVE/ACT epilogue
reads PSUM and writes the output shard.
"""

import os
import sys
import types

sys.path.insert(0, "/opt/trn_rl_repo")

import numpy as np

# trace=True under axon needs antenv.axon_hooks, absent from this image.
# Register the same ctypes-backed NTFF hook trn_boot would have installed.
if "antenv.axon_hooks" not in sys.modules:
    _m = types.ModuleType("antenv.axon_hooks")
    _m._hook = None

    def _set_hook(h):
        _m._hook = h

    def _get_hook():
        return _m._hook

    _m.set_axon_ntff_profile_hook = _set_hook
    _m.get_axon_ntff_profile_hook = _get_hook
    sys.modules["antenv.axon_hooks"] = _m
    try:
        from trn_agent_boot.trn_boot import _ntff_profile_via_ctypes

        _m.set_axon_ntff_profile_hook(
            _ntff_profile_via_ctypes("/opt/axon/libaxon_pjrt.so")
        )
    except Exception:
        pass

import concourse.bacc as bacc
import concourse.tile as tile
from concourse import mybir
from concourse.bass_utils import run_bass_kernel_spmd

N_CORES = 8
B, D, H = 8192, 1024, 1024
BL = B // N_CORES  # batch rows per core
P = 128  # partitions
NF = 512  # matmul free dim (one PSUM bank of fp32)
KD = D // P  # k-tiles per 1024 contraction
MT = BL // P  # batch m-tiles per core
F32 = mybir.dt.float32
F32R = mybir.dt.float32r

_CACHE = {}


def _build_program(with_bias):
    nc = bacc.Bacc("TRN2", target_bir_lowering=False, debug=False)

    xT = nc.declare_dram_parameter("xT", [D, BL], F32R, isOutput=False)
    sT = nc.declare_dram_parameter("sT", [H, BL], F32R, isOutput=False)
    st = nc.declare_dram_parameter("st", [BL, H], F32, isOutput=False)
    wzr = nc.declare_dram_parameter("wzr", [D + H, 2 * H], F32R, isOutput=False)
    wx = nc.declare_dram_parameter("wx", [D, H], F32R, isOutput=False)
    wh = nc.declare_dram_parameter("wh", [H, H], F32R, isOutput=False)
    if with_bias:
        bzr = nc.declare_dram_parameter("bzr", [1, 2 * H], F32R, isOutput=False)
        bx = nc.declare_dram_parameter("bx", [1, H], F32R, isOutput=False)
    out = nc.declare_dram_parameter("out", [BL, H], F32, isOutput=True)

    with tile.TileContext(nc) as tc:
        with (
            tc.tile_pool(name="acts", bufs=1) as acts,
            tc.tile_pool(name="stash", bufs=1) as stash,
            tc.tile_pool(name="wpool", bufs=22) as wpool,
            tc.tile_pool(name="stp", bufs=3) as stp,
            tc.tile_pool(name="tmp", bufs=3) as tmp,
            tc.tile_pool(name="small", bufs=1) as small,
            tc.tile_pool(name="ps", bufs=8, space="PSUM") as ps,
        ):
            warm_sb = small.tile([1, P], F32, tag="warm_sb")
            nc.vector.memset(warm_sb, 1.0)
            warm_ps = ps.tile([P, P], F32, tag="ps", name="warm_ps")
            for i in range(45):
                nc.tensor.matmul(warm_ps, warm_sb, warm_sb, start=True, stop=True)

            if with_bias:
                ones = small.tile([1, P], F32R, tag="ones")
                nc.vector.memset(ones, 1.0)
                bzr_sb = small.tile([1, 2 * H], F32R, tag="bzr")
                nc.sync.dma_start(out=bzr_sb, in_=bzr.ap())
                bx_sb = small.tile([1, H], F32R, tag="bx")
                nc.sync.dma_start(out=bx_sb, in_=bx.ap())

            # Resident transposed activations: [K, B_local] k-tiles.
            # DMAs are emitted inside the g=0 weight loop below, interleaved
            # k-wise with the first weight block, so the first matmuls'
            # dependencies land early.
            HB = BL // 2
            xT_t = [
                [acts.tile([P, HB], F32R, tag=f"xT{k}_{h}", name=f"xT{k}_{h}") for h in range(2)]
                for k in range(KD)
            ]
            sT_t = [
                [acts.tile([P, HB], F32R, tag=f"sT{k}_{h}", name=f"sT{k}_{h}") for h in range(2)]
                for k in range(KD)
            ]

            def load_act(k, h):
                if k < KD:
                    nc.sync.dma_start(
                        out=xT_t[k][h],
                        in_=xT.ap()[k * P : (k + 1) * P, h * HB : (h + 1) * HB],
                    )
                else:
                    kk = k - KD
                    nc.sync.dma_start(
                        out=sT_t[kk][h],
                        in_=sT.ap()[kk * P : (kk + 1) * P, h * HB : (h + 1) * HB],
                    )

            def act_slice(k, m):
                t = xT_t[k] if k < KD else sT_t[k - KD]
                h, r = divmod(m, 4)
                return t[h][:, r * P : (r + 1) * P]

            # Sigmoid outputs stashed until the final gate pass.
            z_st = [stash.tile([P, H], F32, tag=f"z{m}", name=f"z{m}") for m in range(MT)]
            r_st = [stash.tile([P, H], F32, tag=f"r{m}", name=f"r{m}") for m in range(MT)]

            # ---- Phase 1: fused z/r GEMM, K=2048, N=2048 ----
            # k-outer / m-inner over half-groups of 4 PSUM banks: the first
            # matmul only needs one weight tile and one actT tile, and each
            # half-group's sigmoids drain while the other half computes.
            for g in range(4):  # 512-wide column block of the 2048 zr space
                wt = []
                for k in range(2 * KD):
                    w = wpool.tile([P, NF], F32R, tag="w", name="w")
                    nc.sync.dma_start(
                        out=w,
                        in_=wzr.ap()[
                            k * P : (k + 1) * P, g * NF : (g + 1) * NF
                        ],
                    )
                    wt.append(w)
                    if g == 0:
                        load_act(k, 0)
                if g == 0:
                    for k in range(2 * KD):
                        load_act(k, 1)
                for half in range(2):
                    accs = []
                    for mi in range(4):
                        acc = ps.tile([P, NF], F32, tag="ps", name="acc")
                        accs.append(acc)
                        if with_bias:
                            nc.tensor.matmul(
                                acc,
                                ones,
                                bzr_sb[:, g * NF : (g + 1) * NF],
                                start=True,
                                stop=False,
                            )
                    for k in range(2 * KD):
                        for mi in range(4):
                            m = half * 4 + mi
                            lhsT = act_slice(k, m)
                            nc.tensor.matmul(
                                accs[mi],
                                lhsT,
                                wt[k],
                                start=(k == 0 and not with_bias),
                                stop=(k == 2 * KD - 1),
                            )
                    for mi in range(4):
                        m = half * 4 + mi
                        dst = (z_st if g < 2 else r_st)[m][
                            :, (g % 2) * NF : (g % 2 + 1) * NF
                        ]
                        nc.scalar.activation(
                            dst, accs[mi], mybir.ActivationFunctionType.Sigmoid
                        )

            # ---- Phase 2: xh & hh GEMMs + fused gate epilogue ----
            for c in range(2):  # 512-wide column block of H
                csl = slice(c * NF, (c + 1) * NF)
                wxt, wht = [], []
                for k in range(KD):
                    w = wpool.tile([P, NF], F32R, tag="w", name="w")
                    nc.sync.dma_start(
                        out=w, in_=wx.ap()[k * P : (k + 1) * P, csl]
                    )
                    wxt.append(w)
                for k in range(KD):
                    w = wpool.tile([P, NF], F32R, tag="w", name="w")
                    nc.sync.dma_start(
                        out=w, in_=wh.ap()[k * P : (k + 1) * P, csl]
                    )
                    wht.append(w)
                for m in range(MT):
                    msl = slice(m * P, (m + 1) * P)
                    st_t = stp.tile([P, NF], F32, tag="st", name="st_t")
                    nc.sync.dma_start(out=st_t, in_=st.ap()[msl, csl])

                    phh = ps.tile([P, NF], F32, tag="ps", name="phh")
                    for k in range(KD):
                        nc.tensor.matmul(
                            phh,
                            act_slice(KD + k, m),
                            wht[k],
                            start=(k == 0),
                            stop=(k == KD - 1),
                        )
                    pxh = ps.tile([P, NF], F32, tag="ps", name="pxh")
                    if with_bias:
                        nc.tensor.matmul(
                            pxh, ones, bx_sb[:, csl], start=True, stop=False
                        )
                    for k in range(KD):
                        nc.tensor.matmul(
                            pxh,
                            act_slice(k, m),
                            wxt[k],
                            start=(k == 0 and not with_bias),
                            stop=(k == KD - 1),
                        )

                    # h_ = tanh(xh + hh*r); hid = h_ + z*(state - h_)
                    # Last two units run in 256-col chunks to halve the
                    # post-matmul drain chain at kernel end.
                    t = tmp.tile([P, NF], F32, tag="t", name="t")
                    h = tmp.tile([P, NF], F32, tag="h", name="h")
                    nchunk = 2 if (c == 1 and m >= MT - 2) else 1
                    cw = NF // nchunk
                    for q in range(nchunk):
                        qs = slice(q * cw, (q + 1) * cw)
                        nc.vector.tensor_mul(
                            t[:, qs], phh[:, qs], r_st[m][:, c * NF + q * cw : c * NF + (q + 1) * cw]
                        )
                        nc.vector.tensor_add(t[:, qs], t[:, qs], pxh[:, qs])
                        nc.scalar.activation(
                            h[:, qs], t[:, qs], mybir.ActivationFunctionType.Tanh
                        )
                        nc.vector.tensor_sub(st_t[:, qs], st_t[:, qs], h[:, qs])
                        nc.vector.tensor_mul(
                            st_t[:, qs], st_t[:, qs], z_st[m][:, c * NF + q * cw : c * NF + (q + 1) * cw]
                        )
                        nc.vector.tensor_add(t[:, qs], h[:, qs], st_t[:, qs])
                        nc.sync.dma_start(
                            out=out.ap()[msl, c * NF + q * cw : c * NF + (q + 1) * cw],
                            in_=t[:, qs],
                        )

    nc.compile()
    return nc


def _get_program(with_bias):
    key = ("nc", with_bias)
    if key not in _CACHE:
        _CACHE[key] = _build_program(with_bias)
    return _CACHE[key]


def kernel(inp, state, wx, bx, wh, wr, ur, uz, wz, br, bz):
    inp = np.asarray(inp, dtype=np.float32)
    state = np.asarray(state, dtype=np.float32)
    w_zr = np.block(
        [
            [np.asarray(wz, np.float32), np.asarray(wr, np.float32)],
            [np.asarray(uz, np.float32), np.asarray(ur, np.float32)],
        ]
    )
    w_x = np.ascontiguousarray(np.asarray(wx, np.float32))
    w_h = np.ascontiguousarray(np.asarray(wh, np.float32))
    b_zr = np.concatenate(
        [np.asarray(bz, np.float32), np.asarray(br, np.float32)]
    )[None, :]
    b_x = np.ascontiguousarray(np.asarray(bx, np.float32))[None, :]
    xT = np.ascontiguousarray(inp.T)
    sT = np.ascontiguousarray(state.T)

    with_bias = bool(np.any(b_zr) or np.any(b_x))
    in_maps = []
    for c in range(N_CORES):
        sl = slice(c * BL, (c + 1) * BL)
        im = {
            "xT": np.ascontiguousarray(xT[:, sl]),
            "sT": np.ascontiguousarray(sT[:, sl]),
            "st": np.ascontiguousarray(state[sl]),
            "wzr": w_zr,
            "wx": w_x,
            "wh": w_h,
        }
        if with_bias:
            im["bzr"] = b_zr
            im["bx"] = b_x
        in_maps.append(im)

    nc = _get_program(with_bias)
    trace = bool(int(os.environ.get("GRU_TRACE", "0")))
    res = run_bass_kernel_spmd(nc, in_maps, list(range(N_CORES)), trace=trace)
    if trace:
        _CACHE["last_exec_time_ns"] = res.exec_time_ns
        _CACHE["last_results"] = res
    return np.concatenate([res.results[c]["out"] for c in range(N_CORES)], axis=0)


# revision 12
# speedup vs baseline: 1.0517x; 1.0517x over previous
"""GRU cell kernel for Trainium2, data-parallel over batch across 8 NeuronCores.

Reference computation (B=8192, D=H=1024), per batch row:
    z = sigmoid(inp@wz + state@uz + bz)
    r = sigmoid(inp@wr + state@ur + br)
    h_ = tanh(inp@wx + bx + (state@wh) * r)
    hid = (1-z)*h_ + state*z

Strategy: each core takes a 1024-row batch shard. The z/r projections fuse
into one [1024,2048]@[2048,2048] GEMM (act = [inp|state], W = [[wz,wr],[uz,ur]]).
xh and hh stay separate GEMMs ([1024,1024]@[1024,1024] each) because hh is
gated by r before the sum. Activations are shipped pre-transposed from the
host ([K,B] layout) so they can be the stationary matmul operand; weights
stream from HBM as the moving operand. Biases enter the PSUM accumulation
group as a K=1 rank-one matmul against a ones-row. A fused DVE/ACT epilogue
reads PSUM and writes the output shard.
"""

import os
import sys
import types

sys.path.insert(0, "/opt/trn_rl_repo")

import numpy as np

# trace=True under axon needs antenv.axon_hooks, absent from this image.
# Register the same ctypes-backed NTFF hook trn_boot would have installed.
if "antenv.axon_hooks" not in sys.modules:
    _m = types.ModuleType("antenv.axon_hooks")
    _m._hook = None

    def _set_hook(h):
        _m._hook = h

    def _get_hook():
        return _m._hook

    _m.set_axon_ntff_profile_hook = _set_hook
    _m.get_axon_ntff_profile_hook = _get_hook
    sys.modules["antenv.axon_hooks"] = _m
    try:
        from trn_agent_boot.trn_boot import _ntff_profile_via_ctypes

        _m.set_axon_ntff_profile_hook(
            _ntff_profile_via_ctypes("/opt/axon/libaxon_pjrt.so")
        )
    except Exception:
        pass

import concourse.bacc as bacc
import concourse.tile as tile
from concourse import mybir
from concourse.bass_utils import run_bass_kernel_spmd

N_CORES = 8
B, D, H = 8192, 1024, 1024
BL = B // N_CORES  # batch rows per core
P = 128  # partitions
NF = 512  # matmul free dim (one PSUM bank of fp32)
KD = D // P  # k-tiles per 1024 contraction
MT = BL // P  # batch m-tiles per core
F32 = mybir.dt.float32
F32R = mybir.dt.float32r

_CACHE = {}


def _build_program(with_bias):
    nc = bacc.Bacc("TRN2", target_bir_lowering=False, debug=False)

    xT = nc.declare_dram_parameter("xT", [D, BL], F32R, isOutput=False)
    sT = nc.declare_dram_parameter("sT", [H, BL], F32R, isOutput=False)
    st = nc.declare_dram_parameter("st", [BL, H], F32, isOutput=False)
    wzr = nc.declare_dram_parameter("wzr", [D + H, 2 * H], F32R, isOutput=False)
    wx = nc.declare_dram_parameter("wx", [D, H], F32R, isOutput=False)
    wh = nc.declare_dram_parameter("wh", [H, H], F32R, isOutput=False)
    if with_bias:
        bzr = nc.declare_dram_parameter("bzr", [1, 2 * H], F32R, isOutput=False)
        bx = nc.declare_dram_parameter("bx", [1, H], F32R, isOutput=False)
    out = nc.declare_dram_parameter("out", [BL, H], F32, isOutput=True)

    with tile.TileContext(nc) as tc:
        with (
            tc.tile_pool(name="acts", bufs=1) as acts,
            tc.tile_pool(name="stash", bufs=1) as stash,
            tc.tile_pool(name="wpool", bufs=22) as wpool,
            tc.tile_pool(name="stp", bufs=3) as stp,
            tc.tile_pool(name="tmp", bufs=3) as tmp,
            tc.tile_pool(name="small", bufs=1) as small,
            tc.tile_pool(name="ps", bufs=8, space="PSUM") as ps,
        ):
            warm_sb = small.tile([P, 2 * P], F32, tag="warm_sb")
            nc.vector.memset(warm_sb, 0.0)
            warm_ps = ps.tile([P, 2 * P], F32, tag="ps", name="warm_ps")
            for i in range(10):
                nc.tensor.matmul(
                    warm_ps, warm_sb[:, :P], warm_sb, start=True, stop=True
                )

            if with_bias:
                ones = small.tile([1, P], F32R, tag="ones")
                nc.vector.memset(ones, 1.0)
                bzr_sb = small.tile([1, 2 * H], F32R, tag="bzr")
                nc.sync.dma_start(out=bzr_sb, in_=bzr.ap())
                bx_sb = small.tile([1, H], F32R, tag="bx")
                nc.sync.dma_start(out=bx_sb, in_=bx.ap())

            # Resident transposed activations: [K, B_local] k-tiles.
            # DMAs are emitted inside the g=0 weight loop below, interleaved
            # k-wise with the first weight block, so the first matmuls'
            # dependencies land early.
            HB = BL // 2
            xT_t = [
                [acts.tile([P, HB], F32R, tag=f"xT{k}_{h}", name=f"xT{k}_{h}") for h in range(2)]
                for k in range(KD)
            ]
            sT_t = [
                [acts.tile([P, HB], F32R, tag=f"sT{k}_{h}", name=f"sT{k}_{h}") for h in range(2)]
                for k in range(KD)
            ]

            def load_act(k, h):
                if k < KD:
                    nc.sync.dma_start(
                        out=xT_t[k][h],
                        in_=xT.ap()[k * P : (k + 1) * P, h * HB : (h + 1) * HB],
                    )
                else:
                    kk = k - KD
                    nc.sync.dma_start(
                        out=sT_t[kk][h],
                        in_=sT.ap()[kk * P : (kk + 1) * P, h * HB : (h + 1) * HB],
                    )

            def act_slice(k, m):
                t = xT_t[k] if k < KD else sT_t[k - KD]
                h, r = divmod(m, 4)
                return t[h][:, r * P : (r + 1) * P]

            # Sigmoid outputs stashed until the final gate pass.
            z_st = [stash.tile([P, H], F32, tag=f"z{m}", name=f"z{m}") for m in range(MT)]
            r_st = [stash.tile([P, H], F32, tag=f"r{m}", name=f"r{m}") for m in range(MT)]

            # ---- Phase 1: fused z/r GEMM, K=2048, N=2048 ----
            # k-outer / m-inner over half-groups of 4 PSUM banks: the first
            # matmul only needs one weight tile and one actT tile, and each
            # half-group's sigmoids drain while the other half computes.
            for g in range(4):  # 512-wide column block of the 2048 zr space
                wt = []
                for k in range(2 * KD):
                    w = wpool.tile([P, NF], F32R, tag="w", name="w")
                    nc.sync.dma_start(
                        out=w,
                        in_=wzr.ap()[
                            k * P : (k + 1) * P, g * NF : (g + 1) * NF
                        ],
                    )
                    wt.append(w)
                    if g == 0:
                        load_act(k, 0)
                if g == 0:
                    for k in range(2 * KD):
                        load_act(k, 1)
                for half in range(2):
                    accs = []
                    for mi in range(4):
                        acc = ps.tile([P, NF], F32, tag="ps", name="acc")
                        accs.append(acc)
                        if with_bias:
                            nc.tensor.matmul(
                                acc,
                                ones,
                                bzr_sb[:, g * NF : (g + 1) * NF],
                                start=True,
                                stop=False,
                            )
                    for k in range(2 * KD):
                        for mi in range(4):
                            m = half * 4 + mi
                            lhsT = act_slice(k, m)
                            nc.tensor.matmul(
                                accs[mi],
                                lhsT,
                                wt[k],
                                start=(k == 0 and not with_bias),
                                stop=(k == 2 * KD - 1),
                            )
                    for mi in range(4):
                        m = half * 4 + mi
                        dst = (z_st if g < 2 else r_st)[m][
                            :, (g % 2) * NF : (g % 2 + 1) * NF
                        ]
                        nc.scalar.activation(
                            dst, accs[mi], mybir.ActivationFunctionType.Sigmoid
                        )

            # ---- Phase 2: xh & hh GEMMs + fused gate epilogue ----
            for c in range(2):  # 512-wide column block of H
                csl = slice(c * NF, (c + 1) * NF)
                wxt, wht = [], []
                for k in range(KD):
                    w = wpool.tile([P, NF], F32R, tag="w", name="w")
                    nc.sync.dma_start(
                        out=w, in_=wx.ap()[k * P : (k + 1) * P, csl]
                    )
                    wxt.append(w)
                for k in range(KD):
                    w = wpool.tile([P, NF], F32R, tag="w", name="w")
                    nc.sync.dma_start(
                        out=w, in_=wh.ap()[k * P : (k + 1) * P, csl]
                    )
                    wht.append(w)
                for m in range(MT):
                    msl = slice(m * P, (m + 1) * P)
                    st_t = stp.tile([P, NF], F32, tag="st", name="st_t")
                    nc.sync.dma_start(out=st_t, in_=st.ap()[msl, csl])

                    phh = ps.tile([P, NF], F32, tag="ps", name="phh")
                    for k in range(KD):
                        nc.tensor.matmul(
                            phh,
                            act_slice(KD + k, m),
                            wht[k],
                            start=(k == 0),
                            stop=(k == KD - 1),
                        )
                    pxh = ps.tile([P, NF], F32, tag="ps", name="pxh")
                    if with_bias:
                        nc.tensor.matmul(
                            pxh, ones, bx_sb[:, csl], start=True, stop=False
                        )
                    for k in range(KD):
                        nc.tensor.matmul(
                            pxh,
                            act_slice(k, m),
                            wxt[k],
                            start=(k == 0 and not with_bias),
                            stop=(k == KD - 1),
                        )

                    # h_ = tanh(xh + hh*r); hid = h_ + z*(state - h_)
                    # Last two units run in 256-col chunks to halve the
                    # post-matmul drain chain at kernel end.
                    t = tmp.tile([P, NF], F32, tag="t", name="t")
                    h = tmp.tile([P, NF], F32, tag="h", name="h")
                    nchunk = 2 if (c == 1 and m >= MT - 2) else 1
                    cw = NF // nchunk
                    for q in range(nchunk):
                        qs = slice(q * cw, (q + 1) * cw)
                        nc.vector.tensor_mul(
                            t[:, qs], phh[:, qs], r_st[m][:, c * NF + q * cw : c * NF + (q + 1) * cw]
                        )
                        nc.vector.tensor_add(t[:, qs], t[:, qs], pxh[:, qs])
                        nc.scalar.activation(
                            h[:, qs], t[:, qs], mybir.ActivationFunctionType.Tanh
                        )
                        nc.vector.tensor_sub(st_t[:, qs], st_t[:, qs], h[:, qs])
                        nc.vector.tensor_mul(
                            st_t[:, qs], st_t[:, qs], z_st[m][:, c * NF + q * cw : c * NF + (q + 1) * cw]
                        )
                        nc.vector.tensor_add(t[:, qs], h[:, qs], st_t[:, qs])
                        nc.sync.dma_start(
                            out=out.ap()[msl, c * NF + q * cw : c * NF + (q + 1) * cw],
                            in_=t[:, qs],
                        )

    nc.compile()
    return nc


def _get_program(with_bias):
    key = ("nc", with_bias)
    if key not in _CACHE:
        _CACHE[key] = _build_program(with_bias)
    return _CACHE[key]


def kernel(inp, state, wx, bx, wh, wr, ur, uz, wz, br, bz):
    inp = np.asarray(inp, dtype=np.float32)
    state = np.asarray(state, dtype=np.float32)
    w_zr = np.block(
        [
            [np.asarray(wz, np.float32), np.asarray(wr, np.float32)],
            [np.asarray(uz, np.float32), np.asarray(ur, np.float32)],
        ]
    )
    w_x = np.ascontiguousarray(np.asarray(wx, np.float32))
    w_h = np.ascontiguousarray(np.asarray(wh, np.float32))
    b_zr = np.concatenate(
        [np.asarray(bz, np.float32), np.asarray(br, np.float32)]
    )[None, :]
    b_x = np.ascontiguousarray(np.asarray(bx, np.float32))[None, :]
    xT = np.ascontiguousarray(inp.T)
    sT = np.ascontiguousarray(state.T)

    with_bias = bool(np.any(b_zr) or np.any(b_x))
    in_maps = []
    for c in range(N_CORES):
        sl = slice(c * BL, (c + 1) * BL)
        im = {
            "xT": np.ascontiguousarray(xT[:, sl]),
            "sT": np.ascontiguousarray(sT[:, sl]),
            "st": np.ascontiguousarray(state[sl]),
            "wzr": w_zr,
            "wx": w_x,
            "wh": w_h,
        }
        if with_bias:
            im["bzr"] = b_zr
            im["bx"] = b_x
        in_maps.append(im)

    nc = _get_program(with_bias)
    trace = bool(int(os.environ.get("GRU_TRACE", "0")))
    res = run_bass_kernel_spmd(nc, in_maps, list(range(N_CORES)), trace=trace)
    if trace:
        _CACHE["last_exec_time_ns"] = res.exec_time_ns
        _CACHE["last_results"] = res
    return np.concatenate([res.results[c]["out"] for c in range(N_CORES)], axis=0)


# revision 13
# speedup vs baseline: 1.0909x; 1.0373x over previous
"""GRU cell kernel for Trainium2, data-parallel over batch across 8 NeuronCores.

Reference computation (B=8192, D=H=1024), per batch row:
    z = sigmoid(inp@wz + state@uz + bz)
    r = sigmoid(inp@wr + state@ur + br)
    h_ = tanh(inp@wx + bx + (state@wh) * r)
    hid = (1-z)*h_ + state*z

Strategy: each core takes a 1024-row batch shard. The z/r projections fuse
into one [1024,2048]@[2048,2048] GEMM (act = [inp|state], W = [[wz,wr],[uz,ur]]).
xh and hh stay separate GEMMs ([1024,1024]@[1024,1024] each) because hh is
gated by r before the sum. Activations are shipped pre-transposed from the
host ([K,B] layout) so they can be the stationary matmul operand; weights
stream from HBM as the moving operand. Biases enter the PSUM accumulation
group as a K=1 rank-one matmul against a ones-row. A fused DVE/ACT epilogue
reads PSUM and writes the output shard.
"""

import os
import sys
import types

sys.path.insert(0, "/opt/trn_rl_repo")

import numpy as np

# trace=True under axon needs antenv.axon_hooks, absent from this image.
# Register the same ctypes-backed NTFF hook trn_boot would have installed.
if "antenv.axon_hooks" not in sys.modules:
    _m = types.ModuleType("antenv.axon_hooks")
    _m._hook = None

    def _set_hook(h):
        _m._hook = h

    def _get_hook():
        return _m._hook

    _m.set_axon_ntff_profile_hook = _set_hook
    _m.get_axon_ntff_profile_hook = _get_hook
    sys.modules["antenv.axon_hooks"] = _m
    try:
        from trn_agent_boot.trn_boot import _ntff_profile_via_ctypes

        _m.set_axon_ntff_profile_hook(
            _ntff_profile_via_ctypes("/opt/axon/libaxon_pjrt.so")
        )
    except Exception:
        pass

import concourse.bacc as bacc
import concourse.tile as tile
from concourse import mybir
from concourse.bass_utils import run_bass_kernel_spmd

N_CORES = 8
B, D, H = 8192, 1024, 1024
BL = B // N_CORES  # batch rows per core
P = 128  # partitions
NF = 512  # matmul free dim (one PSUM bank of fp32)
KD = D // P  # k-tiles per 1024 contraction
MT = BL // P  # batch m-tiles per core
F32 = mybir.dt.float32
F32R = mybir.dt.float32r

_CACHE = {}


def _build_program(with_bias):
    nc = bacc.Bacc("TRN2", target_bir_lowering=False, debug=False)

    xT = nc.declare_dram_parameter("xT", [D, BL], F32R, isOutput=False)
    sT = nc.declare_dram_parameter("sT", [H, BL], F32R, isOutput=False)
    st = nc.declare_dram_parameter("st", [BL, H], F32, isOutput=False)
    wzr = nc.declare_dram_parameter("wzr", [D + H, 2 * H], F32R, isOutput=False)
    wx = nc.declare_dram_parameter("wx", [D, H], F32R, isOutput=False)
    wh = nc.declare_dram_parameter("wh", [H, H], F32R, isOutput=False)
    if with_bias:
        bzr = nc.declare_dram_parameter("bzr", [1, 2 * H], F32R, isOutput=False)
        bx = nc.declare_dram_parameter("bx", [1, H], F32R, isOutput=False)
    out = nc.declare_dram_parameter("out", [BL, H], F32, isOutput=True)

    with tile.TileContext(nc) as tc:
        with (
            tc.tile_pool(name="acts", bufs=1) as acts,
            tc.tile_pool(name="stash", bufs=1) as stash,
            tc.tile_pool(name="wpool", bufs=32) as wpool,
            tc.tile_pool(name="stp", bufs=3) as stp,
            tc.tile_pool(name="tmp", bufs=3) as tmp,
            tc.tile_pool(name="small", bufs=1) as small,
            tc.tile_pool(name="ps", bufs=8, space="PSUM") as ps,
        ):
            # A few K=128 fp32 matmuls on scratch data keep the PE busy while
            # the input DMAs land, so HAM is un-throttled when real work hits.
            warm_sb = small.tile([P, 2 * P], F32, tag="warm_sb")
            nc.vector.memset(warm_sb, 0.0)
            warm_ps = ps.tile([P, 2 * P], F32, tag="ps", name="warm_ps")
            for i in range(8):
                nc.tensor.matmul(
                    warm_ps, warm_sb[:, :P], warm_sb, start=True, stop=True
                )

            if with_bias:
                ones = small.tile([1, P], F32R, tag="ones")
                nc.vector.memset(ones, 1.0)
                bzr_sb = small.tile([1, 2 * H], F32R, tag="bzr")
                nc.sync.dma_start(out=bzr_sb, in_=bzr.ap())
                bx_sb = small.tile([1, H], F32R, tag="bx")
                nc.sync.dma_start(out=bx_sb, in_=bx.ap())

            # Resident transposed activations, split into batch-half tiles so
            # the first half-group's matmuls only wait on half the data.
            HB = BL // 2
            xT_t = [
                [acts.tile([P, HB], F32R, tag=f"xT{k}_{h}", name=f"xT{k}_{h}") for h in range(2)]
                for k in range(KD)
            ]
            sT_t = [
                [acts.tile([P, HB], F32R, tag=f"sT{k}_{h}", name=f"sT{k}_{h}") for h in range(2)]
                for k in range(KD)
            ]

            def load_act(k, h):
                if k < KD:
                    nc.sync.dma_start(
                        out=xT_t[k][h],
                        in_=xT.ap()[k * P : (k + 1) * P, h * HB : (h + 1) * HB],
                    )
                else:
                    kk = k - KD
                    nc.sync.dma_start(
                        out=sT_t[kk][h],
                        in_=sT.ap()[kk * P : (kk + 1) * P, h * HB : (h + 1) * HB],
                    )

            def act_slice(k, m):
                t = xT_t[k] if k < KD else sT_t[k - KD]
                h, r = divmod(m, 4)
                return t[h][:, r * P : (r + 1) * P]

            # Half-column sigmoid stashes, reused across the two c-rounds.
            z_st = [stash.tile([P, NF], F32, tag=f"z{m}", name=f"z{m}") for m in range(MT)]
            r_st = [stash.tile([P, NF], F32, tag=f"r{m}", name=f"r{m}") for m in range(MT)]

            def zr_block(gcol, dst, first):
                """One 512-col block of the fused z/r GEMM: K=2048, k-outer /
                m-inner over half-groups of 4 PSUM banks; sigmoid into dst."""
                wt = []
                for k in range(2 * KD):
                    w = wpool.tile([P, NF], F32R, tag="w", name="w")
                    nc.sync.dma_start(
                        out=w,
                        in_=wzr.ap()[
                            k * P : (k + 1) * P, gcol * NF : (gcol + 1) * NF
                        ],
                    )
                    wt.append(w)
                    if first:
                        load_act(k, 0)
                if first:
                    for k in range(2 * KD):
                        load_act(k, 1)
                for half in range(2):
                    accs = []
                    for mi in range(4):
                        acc = ps.tile([P, NF], F32, tag="ps", name="acc")
                        accs.append(acc)
                        if with_bias:
                            nc.tensor.matmul(
                                acc,
                                ones,
                                bzr_sb[:, gcol * NF : (gcol + 1) * NF],
                                start=True,
                                stop=False,
                            )
                    for k in range(2 * KD):
                        for mi in range(4):
                            m = half * 4 + mi
                            nc.tensor.matmul(
                                accs[mi],
                                act_slice(k, m),
                                wt[k],
                                start=(k == 0 and not with_bias),
                                stop=(k == 2 * KD - 1),
                            )
                    for mi in range(4):
                        m = half * 4 + mi
                        nc.scalar.activation(
                            dst[m], accs[mi], mybir.ActivationFunctionType.Sigmoid
                        )

            for c in range(2):  # 512-wide column block of H
                csl = slice(c * NF, (c + 1) * NF)
                zr_block(c, z_st, first=(c == 0))       # z columns c*512..
                zr_block(2 + c, r_st, first=False)      # r columns c*512..

                # xh & hh GEMMs + fused gate epilogue for this column block
                wxt, wht = [], []
                for k in range(KD):
                    w = wpool.tile([P, NF], F32R, tag="w", name="w")
                    nc.sync.dma_start(
                        out=w, in_=wx.ap()[k * P : (k + 1) * P, csl]
                    )
                    wxt.append(w)
                for k in range(KD):
                    w = wpool.tile([P, NF], F32R, tag="w", name="w")
                    nc.sync.dma_start(
                        out=w, in_=wh.ap()[k * P : (k + 1) * P, csl]
                    )
                    wht.append(w)
                for m in range(MT):
                    msl = slice(m * P, (m + 1) * P)
                    st_t = stp.tile([P, NF], F32, tag="st", name="st_t")
                    nc.sync.dma_start(out=st_t, in_=st.ap()[msl, csl])

                    phh = ps.tile([P, NF], F32, tag="ps", name="phh")
                    for k in range(KD):
                        nc.tensor.matmul(
                            phh,
                            act_slice(KD + k, m),
                            wht[k],
                            start=(k == 0),
                            stop=(k == KD - 1),
                        )
                    pxh = ps.tile([P, NF], F32, tag="ps", name="pxh")
                    if with_bias:
                        nc.tensor.matmul(
                            pxh, ones, bx_sb[:, csl], start=True, stop=False
                        )
                    for k in range(KD):
                        nc.tensor.matmul(
                            pxh,
                            act_slice(k, m),
                            wxt[k],
                            start=(k == 0 and not with_bias),
                            stop=(k == KD - 1),
                        )

                    # h_ = tanh(xh + hh*r); hid = h_ + z*(state - h_)
                    # The last two units run in 256-col chunks to halve the
                    # post-matmul drain chain at kernel end.
                    t = tmp.tile([P, NF], F32, tag="t", name="t")
                    h = tmp.tile([P, NF], F32, tag="h", name="h")
                    nchunk = 2 if (c == 1 and m >= MT - 2) else 1
                    cw = NF // nchunk
                    for q in range(nchunk):
                        qs = slice(q * cw, (q + 1) * cw)
                        nc.vector.tensor_mul(t[:, qs], phh[:, qs], r_st[m][:, qs])
                        nc.vector.tensor_add(t[:, qs], t[:, qs], pxh[:, qs])
                        nc.scalar.activation(
                            h[:, qs], t[:, qs], mybir.ActivationFunctionType.Tanh
                        )
                        nc.vector.tensor_sub(st_t[:, qs], st_t[:, qs], h[:, qs])
                        nc.vector.tensor_mul(st_t[:, qs], st_t[:, qs], z_st[m][:, qs])
                        nc.vector.tensor_add(t[:, qs], h[:, qs], st_t[:, qs])
                        nc.sync.dma_start(
                            out=out.ap()[msl, c * NF + q * cw : c * NF + (q + 1) * cw],
                            in_=t[:, qs],
                        )

    nc.compile()
    return nc


def _get_program(with_bias):
    key = ("nc", with_bias)
    if key not in _CACHE:
        _CACHE[key] = _build_program(with_bias)
    return _CACHE[key]


def kernel(inp, state, wx, bx, wh, wr, ur, uz, wz, br, bz):
    inp = np.asarray(inp, dtype=np.float32)
    state = np.asarray(state, dtype=np.float32)
    w_zr = np.block(
        [
            [np.asarray(wz, np.float32), np.asarray(wr, np.float32)],
            [np.asarray(uz, np.float32), np.asarray(ur, np.float32)],
        ]
    )
    w_x = np.ascontiguousarray(np.asarray(wx, np.float32))
    w_h = np.ascontiguousarray(np.asarray(wh, np.float32))
    b_zr = np.concatenate(
        [np.asarray(bz, np.float32), np.asarray(br, np.float32)]
    )[None, :]
    b_x = np.ascontiguousarray(np.asarray(bx, np.float32))[None, :]
    xT = np.ascontiguousarray(inp.T)
    sT = np.ascontiguousarray(state.T)

    with_bias = bool(np.any(b_zr) or np.any(b_x))
    in_maps = []
    for c in range(N_CORES):
        sl = slice(c * BL, (c + 1) * BL)
        im = {
            "xT": np.ascontiguousarray(xT[:, sl]),
            "sT": np.ascontiguousarray(sT[:, sl]),
            "st": np.ascontiguousarray(state[sl]),
            "wzr": w_zr,
            "wx": w_x,
            "wh": w_h,
        }
        if with_bias:
            im["bzr"] = b_zr
            im["bx"] = b_x
        in_maps.append(im)

    nc = _get_program(with_bias)
    trace = bool(int(os.environ.get("GRU_TRACE", "0")))
    res = run_bass_kernel_spmd(nc, in_maps, list(range(N_CORES)), trace=trace)
    if trace:
        _CACHE["last_exec_time_ns"] = res.exec_time_ns
        _CACHE["last_results"] = res
    return np.concatenate([res.results[c]["out"] for c in range(N_CORES)], axis=0)


# revision 14
# speedup vs baseline: 1.1576x; 1.0611x over previous
"""GRU cell kernel for Trainium2, data-parallel over batch across 8 NeuronCores.

Reference computation (B=8192, D=H=1024), per batch row:
    z = sigmoid(inp@wz + state@uz + bz)
    r = sigmoid(inp@wr + state@ur + br)
    h_ = tanh(inp@wx + bx + (state@wh) * r)
    hid = (1-z)*h_ + state*z

Strategy: each core takes a 1024-row batch shard. The z/r projections fuse
into one [1024,2048]@[2048,2048] GEMM (act = [inp|state], W = [[wz,wr],[uz,ur]]).
xh and hh stay separate GEMMs ([1024,1024]@[1024,1024] each) because hh is
gated by r before the sum. Activations are shipped pre-transposed from the
host ([K,B] layout) so they can be the stationary matmul operand; weights
stream from HBM as the moving operand. Biases enter the PSUM accumulation
group as a K=1 rank-one matmul against a ones-row. A fused DVE/ACT epilogue
reads PSUM and writes the output shard.
"""

import os
import sys
import types

sys.path.insert(0, "/opt/trn_rl_repo")

import numpy as np
import ml_dtypes

# trace=True under axon needs antenv.axon_hooks, absent from this image.
# Register the same ctypes-backed NTFF hook trn_boot would have installed.
if "antenv.axon_hooks" not in sys.modules:
    _m = types.ModuleType("antenv.axon_hooks")
    _m._hook = None

    def _set_hook(h):
        _m._hook = h

    def _get_hook():
        return _m._hook

    _m.set_axon_ntff_profile_hook = _set_hook
    _m.get_axon_ntff_profile_hook = _get_hook
    sys.modules["antenv.axon_hooks"] = _m
    try:
        from trn_agent_boot.trn_boot import _ntff_profile_via_ctypes

        _m.set_axon_ntff_profile_hook(
            _ntff_profile_via_ctypes("/opt/axon/libaxon_pjrt.so")
        )
    except Exception:
        pass

import concourse.bacc as bacc
import concourse.tile as tile
from concourse import mybir
from concourse.bass_utils import run_bass_kernel_spmd

N_CORES = 8
B, D, H = 8192, 1024, 1024
BL = B // N_CORES  # batch rows per core
P = 128  # partitions
NF = 512  # matmul free dim (one PSUM bank of fp32)
KD = D // P  # k-tiles per 1024 contraction
MT = BL // P  # batch m-tiles per core
F32 = mybir.dt.float32
F32R = mybir.dt.float32r
BF16 = mybir.dt.bfloat16

_CACHE = {}


def _build_program(with_bias):
    nc = bacc.Bacc("TRN2", target_bir_lowering=False, debug=False)

    xT = nc.declare_dram_parameter("xT", [D, BL], BF16, isOutput=False)
    sT = nc.declare_dram_parameter("sT", [H, BL], BF16, isOutput=False)
    st = nc.declare_dram_parameter("st", [BL, H], F32, isOutput=False)
    wzr = nc.declare_dram_parameter("wzr", [D + H, 2 * H], BF16, isOutput=False)
    wx = nc.declare_dram_parameter("wx", [D, H], BF16, isOutput=False)
    wh = nc.declare_dram_parameter("wh", [H, H], BF16, isOutput=False)
    if with_bias:
        bzr = nc.declare_dram_parameter("bzr", [1, 2 * H], BF16, isOutput=False)
        bx = nc.declare_dram_parameter("bx", [1, H], BF16, isOutput=False)
    out = nc.declare_dram_parameter("out", [BL, H], F32, isOutput=True)

    with tile.TileContext(nc) as tc:
        with (
            tc.tile_pool(name="acts", bufs=1) as acts,
            tc.tile_pool(name="stash", bufs=1) as stash,
            tc.tile_pool(name="wpool", bufs=32) as wpool,
            tc.tile_pool(name="stp", bufs=3) as stp,
            tc.tile_pool(name="tmp", bufs=3) as tmp,
            tc.tile_pool(name="small", bufs=1) as small,
            tc.tile_pool(name="ps", bufs=8, space="PSUM") as ps,
        ):
            # A few K=128 fp32 matmuls on scratch data keep the PE busy while
            # the input DMAs land, so HAM is un-throttled when real work hits.
            warm_sb = small.tile([P, 2 * P], F32, tag="warm_sb")
            nc.vector.memset(warm_sb, 0.0)
            warm_ps = ps.tile([P, 2 * P], F32, tag="ps", name="warm_ps")
            for i in range(8):
                nc.tensor.matmul(
                    warm_ps, warm_sb[:, :P], warm_sb, start=True, stop=True
                )

            if with_bias:
                ones = small.tile([1, P], BF16, tag="ones")
                nc.vector.memset(ones, 1.0)
                bzr_sb = small.tile([1, 2 * H], BF16, tag="bzr")
                nc.sync.dma_start(out=bzr_sb, in_=bzr.ap())
                bx_sb = small.tile([1, H], BF16, tag="bx")
                nc.sync.dma_start(out=bx_sb, in_=bx.ap())

            # Resident transposed activations, split into batch-half tiles so
            # the first half-group's matmuls only wait on half the data.
            HB = BL // 2
            xT_t = [
                [acts.tile([P, HB], BF16, tag=f"xT{k}_{h}", name=f"xT{k}_{h}") for h in range(2)]
                for k in range(KD)
            ]
            sT_t = [
                [acts.tile([P, HB], BF16, tag=f"sT{k}_{h}", name=f"sT{k}_{h}") for h in range(2)]
                for k in range(KD)
            ]

            def load_act(k, h):
                if k < KD:
                    nc.sync.dma_start(
                        out=xT_t[k][h],
                        in_=xT.ap()[k * P : (k + 1) * P, h * HB : (h + 1) * HB],
                    )
                else:
                    kk = k - KD
                    nc.sync.dma_start(
                        out=sT_t[kk][h],
                        in_=sT.ap()[kk * P : (kk + 1) * P, h * HB : (h + 1) * HB],
                    )

            def act_slice(k, m):
                t = xT_t[k] if k < KD else sT_t[k - KD]
                h, r = divmod(m, 4)
                return t[h][:, r * P : (r + 1) * P]

            # Half-column sigmoid stashes, reused across the two c-rounds.
            z_st = [stash.tile([P, NF], F32, tag=f"z{m}", name=f"z{m}") for m in range(MT)]
            r_st = [stash.tile([P, NF], F32, tag=f"r{m}", name=f"r{m}") for m in range(MT)]

            def zr_block(gcol, dst, first):
                """One 512-col block of the fused z/r GEMM: K=2048, k-outer /
                m-inner over half-groups of 4 PSUM banks; sigmoid into dst."""
                wt = []
                for k in range(2 * KD):
                    w = wpool.tile([P, NF], BF16, tag="w", name="w")
                    nc.sync.dma_start(
                        out=w,
                        in_=wzr.ap()[
                            k * P : (k + 1) * P, gcol * NF : (gcol + 1) * NF
                        ],
                    )
                    wt.append(w)
                    if first:
                        load_act(k, 0)
                if first:
                    for k in range(2 * KD):
                        load_act(k, 1)
                for half in range(2):
                    accs = []
                    for mi in range(4):
                        acc = ps.tile([P, NF], F32, tag="ps", name="acc")
                        accs.append(acc)
                        if with_bias:
                            nc.tensor.matmul(
                                acc,
                                ones,
                                bzr_sb[:, gcol * NF : (gcol + 1) * NF],
                                start=True,
                                stop=False,
                            )
                    for k in range(2 * KD):
                        for mi in range(4):
                            m = half * 4 + mi
                            nc.tensor.matmul(
                                accs[mi],
                                act_slice(k, m),
                                wt[k],
                                start=(k == 0 and not with_bias),
                                stop=(k == 2 * KD - 1),
                            )
                    for mi in range(4):
                        m = half * 4 + mi
                        nc.scalar.activation(
                            dst[m], accs[mi], mybir.ActivationFunctionType.Sigmoid
                        )

            for c in range(2):  # 512-wide column block of H
                csl = slice(c * NF, (c + 1) * NF)
                zr_block(c, z_st, first=(c == 0))       # z columns c*512..
                zr_block(2 + c, r_st, first=False)      # r columns c*512..

                # xh & hh GEMMs + fused gate epilogue for this column block
                wxt, wht = [], []
                for k in range(KD):
                    w = wpool.tile([P, NF], BF16, tag="w", name="w")
                    nc.sync.dma_start(
                        out=w, in_=wx.ap()[k * P : (k + 1) * P, csl]
                    )
                    wxt.append(w)
                for k in range(KD):
                    w = wpool.tile([P, NF], BF16, tag="w", name="w")
                    nc.sync.dma_start(
                        out=w, in_=wh.ap()[k * P : (k + 1) * P, csl]
                    )
                    wht.append(w)
                for m in range(MT):
                    msl = slice(m * P, (m + 1) * P)
                    st_t = stp.tile([P, NF], F32, tag="st", name="st_t")
                    nc.sync.dma_start(out=st_t, in_=st.ap()[msl, csl])

                    phh = ps.tile([P, NF], F32, tag="ps", name="phh")
                    for k in range(KD):
                        nc.tensor.matmul(
                            phh,
                            act_slice(KD + k, m),
                            wht[k],
                            start=(k == 0),
                            stop=(k == KD - 1),
                        )
                    pxh = ps.tile([P, NF], F32, tag="ps", name="pxh")
                    if with_bias:
                        nc.tensor.matmul(
                            pxh, ones, bx_sb[:, csl], start=True, stop=False
                        )
                    for k in range(KD):
                        nc.tensor.matmul(
                            pxh,
                            act_slice(k, m),
                            wxt[k],
                            start=(k == 0 and not with_bias),
                            stop=(k == KD - 1),
                        )

                    # h_ = tanh(xh + hh*r); hid = h_ + z*(state - h_)
                    # The last two units run in 256-col chunks to halve the
                    # post-matmul drain chain at kernel end.
                    t = tmp.tile([P, NF], F32, tag="t", name="t")
                    h = tmp.tile([P, NF], F32, tag="h", name="h")
                    nchunk = 2 if (c == 1 and m >= MT - 2) else 1
                    cw = NF // nchunk
                    for q in range(nchunk):
                        qs = slice(q * cw, (q + 1) * cw)
                        nc.vector.tensor_mul(t[:, qs], phh[:, qs], r_st[m][:, qs])
                        nc.vector.tensor_add(t[:, qs], t[:, qs], pxh[:, qs])
                        nc.scalar.activation(
                            h[:, qs], t[:, qs], mybir.ActivationFunctionType.Tanh
                        )
                        nc.vector.tensor_sub(st_t[:, qs], st_t[:, qs], h[:, qs])
                        nc.vector.tensor_mul(st_t[:, qs], st_t[:, qs], z_st[m][:, qs])
                        nc.vector.tensor_add(t[:, qs], h[:, qs], st_t[:, qs])
                        nc.sync.dma_start(
                            out=out.ap()[msl, c * NF + q * cw : c * NF + (q + 1) * cw],
                            in_=t[:, qs],
                        )

    nc.compile()
    return nc


def _get_program(with_bias):
    key = ("nc", with_bias)
    if key not in _CACHE:
        _CACHE[key] = _build_program(with_bias)
    return _CACHE[key]


def kernel(inp, state, wx, bx, wh, wr, ur, uz, wz, br, bz):
    inp = np.asarray(inp, dtype=np.float32)
    state = np.asarray(state, dtype=np.float32)
    w_zr = np.block(
        [
            [np.asarray(wz, np.float32), np.asarray(wr, np.float32)],
            [np.asarray(uz, np.float32), np.asarray(ur, np.float32)],
        ]
    )
    w_x = np.ascontiguousarray(np.asarray(wx, np.float32))
    w_h = np.ascontiguousarray(np.asarray(wh, np.float32))
    b_zr = np.concatenate(
        [np.asarray(bz, np.float32), np.asarray(br, np.float32)]
    )[None, :]
    b_x = np.ascontiguousarray(np.asarray(bx, np.float32))[None, :]
    xT = np.ascontiguousarray(inp.T)
    sT = np.ascontiguousarray(state.T)

    with_bias = bool(np.any(b_zr) or np.any(b_x))
    in_maps = []
    for c in range(N_CORES):
        sl = slice(c * BL, (c + 1) * BL)
        im = {
            "xT": np.ascontiguousarray(xT[:, sl]).astype(ml_dtypes.bfloat16),
            "sT": np.ascontiguousarray(sT[:, sl]).astype(ml_dtypes.bfloat16),
            "st": np.ascontiguousarray(state[sl]),
            "wzr": w_zr.astype(ml_dtypes.bfloat16),
            "wx": w_x.astype(ml_dtypes.bfloat16),
            "wh": w_h.astype(ml_dtypes.bfloat16),
        }
        if with_bias:
            im["bzr"] = b_zr.astype(ml_dtypes.bfloat16)
            im["bx"] = b_x.astype(ml_dtypes.bfloat16)
        in_maps.append(im)

    nc = _get_program(with_bias)
    trace = bool(int(os.environ.get("GRU_TRACE", "0")))
    res = run_bass_kernel_spmd(nc, in_maps, list(range(N_CORES)), trace=trace)
    if trace:
        _CACHE["last_exec_time_ns"] = res.exec_time_ns
        _CACHE["last_results"] = res
    return np.concatenate([res.results[c]["out"] for c in range(N_CORES)], axis=0)
